# revision 5
# baseline (speedup 1.0000x reference)
"""NF4-dequantize + matmul kernel for Trainium2, 8-core tensor-parallel.

Column-parallel: d_out sharded across 8 cores (512 cols each). Each core
dequantizes its W shard [4096, 512] on device — two-piece deg-5 minimax
polynomial of the nibble code evaluated in fp16 on a centered grid
s = (code&7 - 3.5)/4 (max level error ~2e-3, below the bf16 rounding floor
of W) — then computes x @ W + bias with bf16 PE matmuls accumulating fp32
in PSUM (bias folded in as a K=1 matmul of a ones-row against the bias row).

Host prep is layout-only: reshape/shard, int32->uint8 nibble unpack, nibble
field split + dtype casts (exact), scale broadcast, bf16 cast + transpose
of x so the contraction dim lands on SBUF partitions.
"""
import numpy as np
import ml_dtypes

import concourse.mybir as mybir
from concourse import bacc
from concourse.tile import TileContext
from concourse import bass_utils
from concourse.alu_op_type import AluOpType as alu

f32 = mybir.dt.float32
f16 = mybir.dt.float16
bf16 = mybir.dt.bfloat16
AF = mybir.ActivationFunctionType

D_IN = 4096
D_OUT = 4096
M = 8192
N_CORES = 8
E = D_OUT // N_CORES          # 512 output cols per core
DT = D_IN // 128              # 32 d-tiles (contraction)
MT = M // 128                 # 64 m-tiles
CHUNK = 8                     # m-tiles per PSUM chunk
XTM = CHUNK * 128             # m columns per xt tile (1024)

# deg-5 minimax fits of the NF4 half-tables on the centered grid
# s = (t - 3.5)/4, t = code & 7 (ascending a0..a5). CD = pos - neg.
CN = [-0.3393859921164608, 0.44642664798357934, -0.07513227750148166,
      0.026909689392366973, -0.17323093754904703, 0.1781004133678547]
CD = [0.7281645770833296, -0.031771748297893754, 0.15588036399512067,
      0.0044205674103370285, 0.3232256503332248, -0.029150018237887876]

_NC_CACHE = {}


def _fit_coeffs():
    """Recompute minimax coefficients (reference for the hardcoded values)."""
    nf4 = np.array([-1.0, -0.6961928009986877, -0.5250730514526367,
                    -0.39491748809814453, -0.28444138169288635,
                    -0.18477343022823334, -0.09105003625154495, 0.0,
                    0.07958029955625534, 0.16093020141124725,
                    0.24611230194568634, 0.33791524171829224,
                    0.44070982933044434, 0.5626170039176941,
                    0.7229568362236023, 1.0])
    grid = (np.arange(8.0) - 3.5) / 4.0

    def mm(y):
        w = np.ones(8)
        for _ in range(200):
            c = np.polyfit(grid, y, 5, w=w)
            r = np.abs(np.polyval(c, grid) - y)
            w *= (1 + r / (r.max() + 1e-18))
        return c[::-1]

    cn = mm(nf4[:8])
    cd = mm(nf4[8:]) - cn
    return list(cn), list(cd)


def _build_nc():
    nc = bacc.Bacc("TRN2", target_bir_lowering=False, debug=False, num_devices=1)
    xt = nc.dram_tensor("xt", [D_IN, M], f16, kind="ExternalInput")
    sg = nc.dram_tensor("sg", [D_IN, E], f16, kind="ExternalInput")    # (code&7 - 3.5)/4
    hg = nc.dram_tensor("hg", [D_IN, E], f16, kind="ExternalInput")    # code>>3
    sx = nc.dram_tensor("sx", [D_IN, E], f16, kind="ExternalInput")    # scales, pre-broadcast
    biasr = nc.dram_tensor("biasr", [1, E], f16, kind="ExternalInput")
    out = nc.dram_tensor("out", [M, E], f32, kind="ExternalOutput")

    with TileContext(nc) as tc:
        with tc.tile_pool(name="wres", bufs=1) as wres, \
             tc.tile_pool(name="dq", bufs=2) as dq, \
             tc.tile_pool(name="xp", bufs=4) as xp, \
             tc.tile_pool(name="pp", bufs=8, space="PSUM") as pp, \
             tc.tile_pool(name="op", bufs=4) as op:

            ones_t = wres.tile([1, 128], f16, tag="ones")
            nc.vector.memset(ones_t[:, :], 1.0)
            bias_t = wres.tile([1, E], f16, tag="bias")
            nc.sync.dma_start(bias_t[:, :], biasr[:, :])

            # ---- dequant W: 32 tiles of [128, 512] bf16, resident in SBUF
            wtiles = []
            for d in range(DT):
                r0 = d * 128
                S = dq.tile([128, E], f16, tag="S")
                Hh = dq.tile([128, E], f16, tag="H")
                sx_t = dq.tile([128, E], f16, tag="sx")
                nc.sync.dma_start(S[:, :], sg[r0:r0 + 128, :])
                nc.sync.dma_start(Hh[:, :], hg[r0:r0 + 128, :])
                nc.sync.dma_start(sx_t[:, :], sx[r0:r0 + 128, :])

                t2 = dq.tile([128, E], f16, tag="t2")
                t4 = dq.tile([128, E], f16, tag="t4")
                nc.scalar.activation(t2[:, :], S[:, :], AF.Square)
                nc.scalar.activation(t4[:, :], t2[:, :], AF.Square)
                An = dq.tile([128, E], f16, tag="An")
                Ad = dq.tile([128, E], f16, tag="Ad")
                nc.scalar.activation(An[:, :], S[:, :], AF.Copy,
                                     bias=float(CN[0]), scale=float(CN[1]))
                nc.scalar.activation(Ad[:, :], S[:, :], AF.Copy,
                                     bias=float(CD[0]), scale=float(CD[1]))
                prs = {}
                for nm, (c0, c1) in (("Bn", (CN[2], CN[3])), ("Bd", (CD[2], CD[3]))):
                    p = dq.tile([128, E], f16, tag=nm, name=nm)
                    nc.scalar.activation(p[:, :], S[:, :], AF.Copy,
                                         bias=float(c0), scale=float(c1))
                    prs[nm] = p
                for nm, (c0, c1) in (("Cn", (CN[4], CN[5])), ("Cd", (CD[4], CD[5]))):
                    p = dq.tile([128, E], f16, tag=nm, name=nm)
                    nc.vector.tensor_scalar(p[:, :], S[:, :], float(c1), float(c0),
                                            alu.mult, alu.add)
                    prs[nm] = p

                def estrin(A, B, C, sufx):
                    u1 = dq.tile([128, E], f16, tag="u1" + sufx, name="u1" + sufx)
                    nc.vector.tensor_tensor(u1[:, :], t2[:, :], B[:, :], alu.mult)
                    u2 = dq.tile([128, E], f16, tag="u2" + sufx, name="u2" + sufx)
                    nc.gpsimd.tensor_tensor(u2[:, :], t4[:, :], C[:, :], alu.mult)
                    v = dq.tile([128, E], f16, tag="v" + sufx, name="v" + sufx)
                    nc.vector.tensor_tensor(v[:, :], A[:, :], u1[:, :], alu.add)
                    P = dq.tile([128, E], f16, tag="P" + sufx, name="P" + sufx)
                    nc.vector.tensor_tensor(P[:, :], v[:, :], u2[:, :], alu.add)
                    return P

                Pn = estrin(An, prs["Bn"], prs["Cn"], "n")
                Pd = estrin(Ad, prs["Bd"], prs["Cd"], "d")

                uu = dq.tile([128, E], f16, tag="uu")
                nc.gpsimd.tensor_tensor(uu[:, :], Pd[:, :], Hh[:, :], alu.mult)
                vv = dq.tile([128, E], f16, tag="vv")
                nc.vector.tensor_tensor(vv[:, :], Pn[:, :], uu[:, :], alu.add)

                w_t = wres.tile([128, E], f16, tag=f"w{d}", name=f"w{d}")
                nc.vector.tensor_tensor(w_t[:, :], vv[:, :], sx_t[:, :], alu.mult)
                wtiles.append(w_t)

            # ---- matmul: chunks of 8 m-tiles, K accumulation over 32 d-tiles
            for ch in range(MT // CHUNK):
                m0 = ch * XTM
                ps = [pp.tile([128, E], f32, tag="ps", name=f"ps{ch}_{i}")
                      for i in range(CHUNK)]
                for d in range(DT):
                    xt_t = xp.tile([128, XTM], f16, tag="xt", name="xt")
                    nc.sync.dma_start(xt_t[:, :], xt[d*128:(d+1)*128, m0:m0 + XTM])
                    for mt in range(CHUNK):
                        nc.tensor.matmul(
                            ps[mt][:, :],
                            xt_t[:, mt*128:(mt+1)*128],
                            wtiles[d][:, :],
                            start=(d == 0), stop=False)
                for mt in range(CHUNK):
                    nc.tensor.matmul(ps[mt][:, :], ones_t[:, :], bias_t[:, :],
                                     start=False, stop=True)
                    ot = op.tile([128, E], f32, tag="ot", name="ot")
                    nc.scalar.copy(ot[:, :], ps[mt][:, :])
                    r = m0 + mt * 128
                    nc.sync.dma_start(out[r:r + 128, :], ot[:, :])

    nc.compile()
    return nc


def _get_nc():
    if "nc" not in _NC_CACHE:
        _NC_CACHE["nc"] = _build_nc()
    return _NC_CACHE["nc"]


def _prep_inputs(x, kernel_quantized, kernel_scales, bias):
    X = np.asarray(x, dtype=np.float32).reshape(M, D_IN)
    xt_full = np.ascontiguousarray(X.astype(np.float16).T)  # [D_IN, M]
    kq = np.asarray(kernel_quantized).astype(np.uint8).reshape(D_IN, D_OUT // 2)
    codes = np.empty((D_IN, D_OUT), np.uint8)
    codes[:, 0::2] = kq >> 4
    codes[:, 1::2] = kq & 15
    sg_full = (((codes & 7).astype(np.float16) - np.float16(3.5)) * np.float16(0.25))
    hg_full = (codes >> 3).astype(np.float16)
    sc = np.asarray(kernel_scales, dtype=np.float32).reshape(D_IN, D_OUT // 64)
    sx_full = np.repeat(sc.astype(np.float16), 64, axis=1)          # [D_IN, D_OUT]
    b_full = np.asarray(bias, dtype=np.float32)

    in_maps = []
    for c in range(N_CORES):
        sl = slice(c * E, (c + 1) * E)
        in_maps.append({
            "xt": xt_full,
            "sg": np.ascontiguousarray(sg_full[:, sl]),
            "hg": np.ascontiguousarray(hg_full[:, sl]),
            "sx": np.ascontiguousarray(sx_full[:, sl]),
            "biasr": b_full[sl].astype(np.float16).reshape(1, E),
        })
    return in_maps


def kernel(x, kernel_quantized, kernel_scales, bias, _trace=False, _tmpdir=None):
    in_maps = _prep_inputs(x, kernel_quantized, kernel_scales, bias)
    nc = _get_nc()
    kwargs = {}
    if _trace:
        kwargs = {"trace": True, "tmpdir": _tmpdir}
    res = bass_utils.run_bass_kernel_spmd(
        nc, in_maps, core_ids=list(range(N_CORES)), **kwargs)
    out = np.concatenate([res.results[c]["out"] for c in range(N_CORES)], axis=1)
    out = np.ascontiguousarray(out).reshape(4, 2048, D_OUT).astype(np.float32)
    if _trace:
        return out, res
    return out


# revision 6
# speedup vs baseline: 1.0364x; 1.0364x over previous
"""NF4-dequantize + matmul kernel for Trainium2, 8-core tensor-parallel.

Column-parallel: d_out sharded across 8 cores (512 cols each). Each core
dequantizes its W shard [4096, 512] on device — two-piece deg-5 minimax
polynomial of the nibble code evaluated in fp16 on a centered grid
s = (code&7 - 3.5)/4 (max level error ~2e-3, below the bf16 rounding floor
of W) — then computes x @ W + bias with bf16 PE matmuls accumulating fp32
in PSUM (bias folded in as a K=1 matmul of a ones-row against the bias row).

Host prep is layout-only: reshape/shard, int32->uint8 nibble unpack, nibble
field split + dtype casts (exact), scale broadcast, bf16 cast + transpose
of x so the contraction dim lands on SBUF partitions.
"""
import numpy as np
import ml_dtypes

import concourse.mybir as mybir
from concourse import bacc
from concourse.tile import TileContext
from concourse import bass_utils
from concourse.alu_op_type import AluOpType as alu

f32 = mybir.dt.float32
f16 = mybir.dt.float16
bf16 = mybir.dt.bfloat16
AF = mybir.ActivationFunctionType

D_IN = 4096
D_OUT = 4096
M = 8192
N_CORES = 8
E = D_OUT // N_CORES          # 512 output cols per core
DT = D_IN // 128              # 32 d-tiles (contraction)
MT = M // 128                 # 64 m-tiles
CHUNK = 8                     # m-tiles per PSUM chunk
XTM = CHUNK * 128             # m columns per xt tile (1024)

# deg-5 minimax fits of the NF4 half-tables on the centered grid
# s = (t - 3.5)/4, t = code & 7 (ascending a0..a5). CD = pos - neg.
CN = [-0.3393859921164608, 0.44642664798357934, -0.07513227750148166,
      0.026909689392366973, -0.17323093754904703, 0.1781004133678547]
CD = [0.7281645770833296, -0.031771748297893754, 0.15588036399512067,
      0.0044205674103370285, 0.3232256503332248, -0.029150018237887876]

_NC_CACHE = {}


def _fit_coeffs():
    """Recompute minimax coefficients (reference for the hardcoded values)."""
    nf4 = np.array([-1.0, -0.6961928009986877, -0.5250730514526367,
                    -0.39491748809814453, -0.28444138169288635,
                    -0.18477343022823334, -0.09105003625154495, 0.0,
                    0.07958029955625534, 0.16093020141124725,
                    0.24611230194568634, 0.33791524171829224,
                    0.44070982933044434, 0.5626170039176941,
                    0.7229568362236023, 1.0])
    grid = (np.arange(8.0) - 3.5) / 4.0

    def mm(y):
        w = np.ones(8)
        for _ in range(200):
            c = np.polyfit(grid, y, 5, w=w)
            r = np.abs(np.polyval(c, grid) - y)
            w *= (1 + r / (r.max() + 1e-18))
        return c[::-1]

    cn = mm(nf4[:8])
    cd = mm(nf4[8:]) - cn
    return list(cn), list(cd)


def _build_nc():
    nc = bacc.Bacc("TRN2", target_bir_lowering=False, debug=False, num_devices=1)
    xt = nc.dram_tensor("xt", [D_IN, M], f16, kind="ExternalInput")
    sg = nc.dram_tensor("sg", [D_IN, E], f16, kind="ExternalInput")    # (code&7 - 3.5)/4
    hg = nc.dram_tensor("hg", [D_IN, E], f16, kind="ExternalInput")    # code>>3
    sx = nc.dram_tensor("sx", [D_IN, E], f16, kind="ExternalInput")    # scales, pre-broadcast
    biasr = nc.dram_tensor("biasr", [1, E], f16, kind="ExternalInput")
    out = nc.dram_tensor("out", [M, E], f32, kind="ExternalOutput")

    with TileContext(nc) as tc:
        with tc.tile_pool(name="wres", bufs=1) as wres, \
             tc.tile_pool(name="dq", bufs=4) as dq, \
             tc.tile_pool(name="xp", bufs=6) as xp, \
             tc.tile_pool(name="pp", bufs=8, space="PSUM") as pp, \
             tc.tile_pool(name="op", bufs=6) as op:

            ones_t = wres.tile([1, 128], f16, tag="ones")
            nc.vector.memset(ones_t[:, :], 1.0)
            bias_t = wres.tile([1, E], f16, tag="bias")
            nc.sync.dma_start(bias_t[:, :], biasr[:, :])

            # ---- dequant W: 32 tiles of [128, 512] bf16, resident in SBUF
            wtiles = []
            for d in range(DT):
                r0 = d * 128
                S = dq.tile([128, E], f16, tag="S")
                Hh = dq.tile([128, E], f16, tag="H")
                sx_t = dq.tile([128, E], f16, tag="sx")
                nc.sync.dma_start(S[:, :], sg[r0:r0 + 128, :])
                nc.sync.dma_start(Hh[:, :], hg[r0:r0 + 128, :])
                nc.sync.dma_start(sx_t[:, :], sx[r0:r0 + 128, :])

                t2 = dq.tile([128, E], f16, tag="t2")
                t4 = dq.tile([128, E], f16, tag="t4")
                nc.scalar.activation(t2[:, :], S[:, :], AF.Square)
                nc.scalar.activation(t4[:, :], t2[:, :], AF.Square)
                An = dq.tile([128, E], f16, tag="An")
                Ad = dq.tile([128, E], f16, tag="Ad")
                nc.scalar.activation(An[:, :], S[:, :], AF.Copy,
                                     bias=float(CN[0]), scale=float(CN[1]))
                nc.scalar.activation(Ad[:, :], S[:, :], AF.Copy,
                                     bias=float(CD[0]), scale=float(CD[1]))
                prs = {}
                for nm, (c0, c1) in (("Bn", (CN[2], CN[3])), ("Bd", (CD[2], CD[3]))):
                    p = dq.tile([128, E], f16, tag=nm, name=nm)
                    nc.scalar.activation(p[:, :], S[:, :], AF.Copy,
                                         bias=float(c0), scale=float(c1))
                    prs[nm] = p
                for nm, (c0, c1) in (("Cn", (CN[4], CN[5])), ("Cd", (CD[4], CD[5]))):
                    p = dq.tile([128, E], f16, tag=nm, name=nm)
                    nc.vector.tensor_scalar(p[:, :], S[:, :], float(c1), float(c0),
                                            alu.mult, alu.add)
                    prs[nm] = p

                def estrin(A, B, C, sufx):
                    u1 = dq.tile([128, E], f16, tag="u1" + sufx, name="u1" + sufx)
                    nc.vector.tensor_tensor(u1[:, :], t2[:, :], B[:, :], alu.mult)
                    u2 = dq.tile([128, E], f16, tag="u2" + sufx, name="u2" + sufx)
                    nc.gpsimd.tensor_tensor(u2[:, :], t4[:, :], C[:, :], alu.mult)
                    v = dq.tile([128, E], f16, tag="v" + sufx, name="v" + sufx)
                    nc.vector.tensor_tensor(v[:, :], A[:, :], u1[:, :], alu.add)
                    P = dq.tile([128, E], f16, tag="P" + sufx, name="P" + sufx)
                    nc.vector.tensor_tensor(P[:, :], v[:, :], u2[:, :], alu.add)
                    return P

                Pn = estrin(An, prs["Bn"], prs["Cn"], "n")
                Pd = estrin(Ad, prs["Bd"], prs["Cd"], "d")

                uu = dq.tile([128, E], f16, tag="uu")
                nc.gpsimd.tensor_tensor(uu[:, :], Pd[:, :], Hh[:, :], alu.mult)
                vv = dq.tile([128, E], f16, tag="vv")
                nc.vector.tensor_tensor(vv[:, :], Pn[:, :], uu[:, :], alu.add)

                w_t = wres.tile([128, E], f16, tag=f"w{d}", name=f"w{d}")
                nc.vector.tensor_tensor(w_t[:, :], vv[:, :], sx_t[:, :], alu.mult)
                wtiles.append(w_t)

            # ---- matmul: chunks of 8 m-tiles, K accumulation over 32 d-tiles
            for ch in range(MT // CHUNK):
                m0 = ch * XTM
                ps = [pp.tile([128, E], f32, tag="ps", name=f"ps{ch}_{i}")
                      for i in range(CHUNK)]
                for d in range(DT):
                    xt_t = xp.tile([128, XTM], f16, tag="xt", name="xt")
                    nc.sync.dma_start(xt_t[:, :], xt[d*128:(d+1)*128, m0:m0 + XTM])
                    for mt in range(CHUNK):
                        nc.tensor.matmul(
                            ps[mt][:, :],
                            xt_t[:, mt*128:(mt+1)*128],
                            wtiles[d][:, :],
                            start=(d == 0), stop=False)
                for mt in range(CHUNK):
                    nc.tensor.matmul(ps[mt][:, :], ones_t[:, :], bias_t[:, :],
                                     start=False, stop=True)
                    ot = op.tile([128, E], f32, tag="ot", name="ot")
                    nc.scalar.copy(ot[:, :], ps[mt][:, :])
                    r = m0 + mt * 128
                    nc.sync.dma_start(out[r:r + 128, :], ot[:, :])

    nc.compile()
    return nc


def _get_nc():
    if "nc" not in _NC_CACHE:
        _NC_CACHE["nc"] = _build_nc()
    return _NC_CACHE["nc"]


def _prep_inputs(x, kernel_quantized, kernel_scales, bias):
    X = np.asarray(x, dtype=np.float32).reshape(M, D_IN)
    xt_full = np.ascontiguousarray(X.astype(np.float16).T)  # [D_IN, M]
    kq = np.asarray(kernel_quantized).astype(np.uint8).reshape(D_IN, D_OUT // 2)
    codes = np.empty((D_IN, D_OUT), np.uint8)
    codes[:, 0::2] = kq >> 4
    codes[:, 1::2] = kq & 15
    sg_full = (((codes & 7).astype(np.float16) - np.float16(3.5)) * np.float16(0.25))
    hg_full = (codes >> 3).astype(np.float16)
    sc = np.asarray(kernel_scales, dtype=np.float32).reshape(D_IN, D_OUT // 64)
    sx_full = np.repeat(sc.astype(np.float16), 64, axis=1)          # [D_IN, D_OUT]
    b_full = np.asarray(bias, dtype=np.float32)

    in_maps = []
    for c in range(N_CORES):
        sl = slice(c * E, (c + 1) * E)
        in_maps.append({
            "xt": xt_full,
            "sg": np.ascontiguousarray(sg_full[:, sl]),
            "hg": np.ascontiguousarray(hg_full[:, sl]),
            "sx": np.ascontiguousarray(sx_full[:, sl]),
            "biasr": b_full[sl].astype(np.float16).reshape(1, E),
        })
    return in_maps


def kernel(x, kernel_quantized, kernel_scales, bias, _trace=False, _tmpdir=None):
    in_maps = _prep_inputs(x, kernel_quantized, kernel_scales, bias)
    nc = _get_nc()
    kwargs = {}
    if _trace:
        kwargs = {"trace": True, "tmpdir": _tmpdir}
    res = bass_utils.run_bass_kernel_spmd(
        nc, in_maps, core_ids=list(range(N_CORES)), **kwargs)
    out = np.concatenate([res.results[c]["out"] for c in range(N_CORES)], axis=1)
    out = np.ascontiguousarray(out).reshape(4, 2048, D_OUT).astype(np.float32)
    if _trace:
        return out, res
    return out


# revision 7
# speedup vs baseline: 1.0710x; 1.0334x over previous
"""NF4-dequantize + matmul kernel for Trainium2, 8-core tensor-parallel.

Column-parallel: d_out sharded across 8 cores (512 cols each). Each core
dequantizes its W shard [4096, 512] on device — two-piece deg-5 minimax
polynomial of the nibble code evaluated in fp16 on a centered grid
s = (code&7 - 3.5)/4 (max level error ~2e-3, below the bf16 rounding floor
of W) — then computes x @ W + bias with bf16 PE matmuls accumulating fp32
in PSUM (bias folded in as a K=1 matmul of a ones-row against the bias row).

Host prep is layout-only: reshape/shard, int32->uint8 nibble unpack, nibble
field split + dtype casts (exact), scale broadcast, bf16 cast + transpose
of x so the contraction dim lands on SBUF partitions.
"""
import numpy as np
import ml_dtypes

import concourse.mybir as mybir
from concourse import bacc
from concourse.tile import TileContext
from concourse import bass_utils
from concourse.alu_op_type import AluOpType as alu

f32 = mybir.dt.float32
f16 = mybir.dt.float16
bf16 = mybir.dt.bfloat16
AF = mybir.ActivationFunctionType

D_IN = 4096
D_OUT = 4096
M = 8192
N_CORES = 8
E = D_OUT // N_CORES          # 512 output cols per core
DT = D_IN // 128              # 32 d-tiles (contraction)
MT = M // 128                 # 64 m-tiles
CHUNK = 8                     # m-tiles per PSUM chunk
XTM = CHUNK * 128             # m columns per xt tile (1024)

# deg-5 minimax fits of the NF4 half-tables on the centered grid
# s = (t - 3.5)/4, t = code & 7 (ascending a0..a5). CD = pos - neg.
CN = [-0.3393859921164608, 0.44642664798357934, -0.07513227750148166,
      0.026909689392366973, -0.17323093754904703, 0.1781004133678547]
CD = [0.7281645770833296, -0.031771748297893754, 0.15588036399512067,
      0.0044205674103370285, 0.3232256503332248, -0.029150018237887876]

_NC_CACHE = {}


def _fit_coeffs():
    """Recompute minimax coefficients (reference for the hardcoded values)."""
    nf4 = np.array([-1.0, -0.6961928009986877, -0.5250730514526367,
                    -0.39491748809814453, -0.28444138169288635,
                    -0.18477343022823334, -0.09105003625154495, 0.0,
                    0.07958029955625534, 0.16093020141124725,
                    0.24611230194568634, 0.33791524171829224,
                    0.44070982933044434, 0.5626170039176941,
                    0.7229568362236023, 1.0])
    grid = (np.arange(8.0) - 3.5) / 4.0

    def mm(y):
        w = np.ones(8)
        for _ in range(200):
            c = np.polyfit(grid, y, 5, w=w)
            r = np.abs(np.polyval(c, grid) - y)
            w *= (1 + r / (r.max() + 1e-18))
        return c[::-1]

    cn = mm(nf4[:8])
    cd = mm(nf4[8:]) - cn
    return list(cn), list(cd)


def _build_nc():
    nc = bacc.Bacc("TRN2", target_bir_lowering=False, debug=False, num_devices=1)
    xt = nc.dram_tensor("xt", [D_IN, M], f16, kind="ExternalInput")
    sg = nc.dram_tensor("sg", [D_IN, E], f16, kind="ExternalInput")    # (code&7 - 3.5)/4
    hg = nc.dram_tensor("hg", [D_IN, E], f16, kind="ExternalInput")    # code>>3
    sx = nc.dram_tensor("sx", [D_IN, E], f16, kind="ExternalInput")    # scales, pre-broadcast
    biasr = nc.dram_tensor("biasr", [1, E], f16, kind="ExternalInput")
    out = nc.dram_tensor("out", [M, E], f32, kind="ExternalOutput")

    with TileContext(nc) as tc:
        with tc.tile_pool(name="wres", bufs=1) as wres, \
             tc.tile_pool(name="dq", bufs=6) as dq, \
             tc.tile_pool(name="xp", bufs=6) as xp, \
             tc.tile_pool(name="pp", bufs=8, space="PSUM") as pp, \
             tc.tile_pool(name="op", bufs=6) as op:

            ones_t = wres.tile([1, 128], f16, tag="ones")
            nc.vector.memset(ones_t[:, :], 1.0)
            bias_t = wres.tile([1, E], f16, tag="bias")
            nc.sync.dma_start(bias_t[:, :], biasr[:, :])

            # ---- dequant W: 32 tiles of [128, 512] f16, resident in SBUF
            # Each tile is assigned a "home" vector engine (DVE or GPSIMD) that
            # runs its whole TT chain; ACT does the shared front-end (squares,
            # A-pairs) for all tiles. This keeps cross-engine hops to ACT->home.
            wtiles = []
            for d in range(DT):
                r0 = d * 128
                home = nc.gpsimd if (d % 5 == 4) else nc.vector
                S = dq.tile([128, E], f16, tag="S", name="S")
                sx_t = dq.tile([128, E], f16, tag="sx", name="sx")
                Hh = dq.tile([128, E], f16, tag="H", name="H")
                nc.sync.dma_start(S[:, :], sg[r0:r0 + 128, :])
                nc.sync.dma_start(Hh[:, :], hg[r0:r0 + 128, :])
                nc.sync.dma_start(sx_t[:, :], sx[r0:r0 + 128, :])

                t2 = dq.tile([128, E], f16, tag="t2", name="t2")
                t4 = dq.tile([128, E], f16, tag="t4", name="t4")
                nc.scalar.activation(t2[:, :], S[:, :], AF.Square)
                nc.scalar.activation(t4[:, :], t2[:, :], AF.Square)
                An = dq.tile([128, E], f16, tag="An", name="An")
                Ad = dq.tile([128, E], f16, tag="Ad", name="Ad")
                nc.scalar.activation(An[:, :], S[:, :], AF.Copy,
                                     bias=float(CN[0]), scale=float(CN[1]))
                nc.scalar.activation(Ad[:, :], S[:, :], AF.Copy,
                                     bias=float(CD[0]), scale=float(CD[1]))
                prs = {}
                for nm, (c0, c1) in (("Bn", (CN[2], CN[3])), ("Bd", (CD[2], CD[3])),
                                     ("Cn", (CN[4], CN[5])), ("Cd", (CD[4], CD[5]))):
                    p = dq.tile([128, E], f16, tag=nm, name=nm)
                    home.tensor_scalar(p[:, :], S[:, :], float(c1), float(c0),
                                       alu.mult, alu.add)
                    prs[nm] = p

                def estrin(A, B, C, sufx):
                    u1 = dq.tile([128, E], f16, tag="u1" + sufx, name="u1" + sufx)
                    home.tensor_tensor(u1[:, :], t2[:, :], B[:, :], alu.mult)
                    u2 = dq.tile([128, E], f16, tag="u2" + sufx, name="u2" + sufx)
                    home.tensor_tensor(u2[:, :], t4[:, :], C[:, :], alu.mult)
                    v = dq.tile([128, E], f16, tag="v" + sufx, name="v" + sufx)
                    home.tensor_tensor(v[:, :], A[:, :], u1[:, :], alu.add)
                    P = dq.tile([128, E], f16, tag="P" + sufx, name="P" + sufx)
                    home.tensor_tensor(P[:, :], v[:, :], u2[:, :], alu.add)
                    return P

                Pn = estrin(An, prs["Bn"], prs["Cn"], "n")
                Pd = estrin(Ad, prs["Bd"], prs["Cd"], "d")

                uu = dq.tile([128, E], f16, tag="uu", name="uu")
                home.tensor_tensor(uu[:, :], Pd[:, :], Hh[:, :], alu.mult)
                vv = dq.tile([128, E], f16, tag="vv", name="vv")
                home.tensor_tensor(vv[:, :], Pn[:, :], uu[:, :], alu.add)

                w_t = wres.tile([128, E], f16, tag=f"w{d}", name=f"w{d}")
                home.tensor_tensor(w_t[:, :], vv[:, :], sx_t[:, :], alu.mult)
                wtiles.append(w_t)

            # ---- matmul: chunks of 8 m-tiles, K accumulation over 32 d-tiles
            for ch in range(MT // CHUNK):
                m0 = ch * XTM
                ps = [pp.tile([128, E], f32, tag="ps", name=f"ps{ch}_{i}")
                      for i in range(CHUNK)]
                for d in range(DT):
                    xt_t = xp.tile([128, XTM], f16, tag="xt", name="xt")
                    nc.sync.dma_start(xt_t[:, :], xt[d*128:(d+1)*128, m0:m0 + XTM])
                    for mt in range(CHUNK):
                        nc.tensor.matmul(
                            ps[mt][:, :],
                            xt_t[:, mt*128:(mt+1)*128],
                            wtiles[d][:, :],
                            start=(d == 0), stop=False)
                for mt in range(CHUNK):
                    nc.tensor.matmul(ps[mt][:, :], ones_t[:, :], bias_t[:, :],
                                     start=False, stop=True)
                    ot = op.tile([128, E], f32, tag="ot", name="ot")
                    nc.scalar.copy(ot[:, :], ps[mt][:, :])
                    r = m0 + mt * 128
                    nc.sync.dma_start(out[r:r + 128, :], ot[:, :])

    nc.compile()
    return nc


def _get_nc():
    if "nc" not in _NC_CACHE:
        _NC_CACHE["nc"] = _build_nc()
    return _NC_CACHE["nc"]


def _prep_inputs(x, kernel_quantized, kernel_scales, bias):
    X = np.asarray(x, dtype=np.float32).reshape(M, D_IN)
    xt_full = np.ascontiguousarray(X.astype(np.float16).T)  # [D_IN, M]
    kq = np.asarray(kernel_quantized).astype(np.uint8).reshape(D_IN, D_OUT // 2)
    codes = np.empty((D_IN, D_OUT), np.uint8)
    codes[:, 0::2] = kq >> 4
    codes[:, 1::2] = kq & 15
    sg_full = (((codes & 7).astype(np.float16) - np.float16(3.5)) * np.float16(0.25))
    hg_full = (codes >> 3).astype(np.float16)
    sc = np.asarray(kernel_scales, dtype=np.float32).reshape(D_IN, D_OUT // 64)
    sx_full = np.repeat(sc.astype(np.float16), 64, axis=1)          # [D_IN, D_OUT]
    b_full = np.asarray(bias, dtype=np.float32)

    in_maps = []
    for c in range(N_CORES):
        sl = slice(c * E, (c + 1) * E)
        in_maps.append({
            "xt": xt_full,
            "sg": np.ascontiguousarray(sg_full[:, sl]),
            "hg": np.ascontiguousarray(hg_full[:, sl]),
            "sx": np.ascontiguousarray(sx_full[:, sl]),
            "biasr": b_full[sl].astype(np.float16).reshape(1, E),
        })
    return in_maps


def kernel(x, kernel_quantized, kernel_scales, bias, _trace=False, _tmpdir=None):
    in_maps = _prep_inputs(x, kernel_quantized, kernel_scales, bias)
    nc = _get_nc()
    kwargs = {}
    if _trace:
        kwargs = {"trace": True, "tmpdir": _tmpdir}
    res = bass_utils.run_bass_kernel_spmd(
        nc, in_maps, core_ids=list(range(N_CORES)), **kwargs)
    out = np.concatenate([res.results[c]["out"] for c in range(N_CORES)], axis=1)
    out = np.ascontiguousarray(out).reshape(4, 2048, D_OUT).astype(np.float32)
    if _trace:
        return out, res
    return out


# revision 8
# speedup vs baseline: 1.0713x; 1.0003x over previous
"""NF4-dequantize + matmul kernel for Trainium2, 8-core tensor-parallel.

Column-parallel: d_out sharded across 8 cores (512 cols each). Each core
dequantizes its W shard [4096, 512] on device — two-piece deg-5 minimax
polynomial of the nibble code evaluated in fp16 on a centered grid
s = (code&7 - 3.5)/4 (max level error ~2e-3, below the bf16 rounding floor
of W) — then computes x @ W + bias with bf16 PE matmuls accumulating fp32
in PSUM (bias folded in as a K=1 matmul of a ones-row against the bias row).

Host prep is layout-only: reshape/shard, int32->uint8 nibble unpack, nibble
field split + dtype casts (exact), scale broadcast, bf16 cast + transpose
of x so the contraction dim lands on SBUF partitions.
"""
import numpy as np
import ml_dtypes

import concourse.mybir as mybir
from concourse import bacc
from concourse.tile import TileContext
from concourse import bass_utils
from concourse.alu_op_type import AluOpType as alu

f32 = mybir.dt.float32
f16 = mybir.dt.float16
bf16 = mybir.dt.bfloat16
AF = mybir.ActivationFunctionType

D_IN = 4096
D_OUT = 4096
M = 8192
N_CORES = 8
E = D_OUT // N_CORES          # 512 output cols per core
DT = D_IN // 128              # 32 d-tiles (contraction)
MT = M // 128                 # 64 m-tiles
CHUNK = 8                     # m-tiles per PSUM chunk
XTM = CHUNK * 128             # m columns per xt tile (1024)

# deg-5 minimax fits of the NF4 half-tables on the centered grid
# s = (t - 3.5)/4, t = code & 7 (ascending a0..a5). CD = pos - neg.
CN = [-0.3393859921164608, 0.44642664798357934, -0.07513227750148166,
      0.026909689392366973, -0.17323093754904703, 0.1781004133678547]
CD = [0.7281645770833296, -0.031771748297893754, 0.15588036399512067,
      0.0044205674103370285, 0.3232256503332248, -0.029150018237887876]

_NC_CACHE = {}


def _fit_coeffs():
    """Recompute minimax coefficients (reference for the hardcoded values)."""
    nf4 = np.array([-1.0, -0.6961928009986877, -0.5250730514526367,
                    -0.39491748809814453, -0.28444138169288635,
                    -0.18477343022823334, -0.09105003625154495, 0.0,
                    0.07958029955625534, 0.16093020141124725,
                    0.24611230194568634, 0.33791524171829224,
                    0.44070982933044434, 0.5626170039176941,
                    0.7229568362236023, 1.0])
    grid = (np.arange(8.0) - 3.5) / 4.0

    def mm(y):
        w = np.ones(8)
        for _ in range(200):
            c = np.polyfit(grid, y, 5, w=w)
            r = np.abs(np.polyval(c, grid) - y)
            w *= (1 + r / (r.max() + 1e-18))
        return c[::-1]

    cn = mm(nf4[:8])
    cd = mm(nf4[8:]) - cn
    return list(cn), list(cd)


def _build_nc():
    nc = bacc.Bacc("TRN2", target_bir_lowering=False, debug=False, num_devices=1)
    xt = nc.dram_tensor("xt", [D_IN, M], f16, kind="ExternalInput")
    sg = nc.dram_tensor("sg", [D_IN, E], f16, kind="ExternalInput")    # (code&7 - 3.5)/4
    hg = nc.dram_tensor("hg", [D_IN, E], f16, kind="ExternalInput")    # code>>3
    sx = nc.dram_tensor("sx", [D_IN, E], f16, kind="ExternalInput")    # scales, pre-broadcast
    biasr = nc.dram_tensor("biasr", [1, E], f16, kind="ExternalInput")
    out = nc.dram_tensor("out", [M, E], f32, kind="ExternalOutput")

    with TileContext(nc) as tc:
        with tc.tile_pool(name="wres", bufs=1) as wres, \
             tc.tile_pool(name="dq", bufs=6) as dq, \
             tc.tile_pool(name="xp", bufs=6) as xp, \
             tc.tile_pool(name="pp", bufs=8, space="PSUM") as pp, \
             tc.tile_pool(name="op", bufs=6) as op:

            ones_t = wres.tile([1, 128], f16, tag="ones")
            nc.vector.memset(ones_t[:, :], 1.0)
            bias_t = wres.tile([1, E], f16, tag="bias")
            nc.sync.dma_start(bias_t[:, :], biasr[:, :])

            # ---- dequant W: 32 tiles of [128, 512] f16, resident in SBUF
            # Each tile is assigned a "home" vector engine (DVE or GPSIMD) that
            # runs its whole TT chain; ACT does the shared front-end (squares,
            # A-pairs) for all tiles. This keeps cross-engine hops to ACT->home.
            wtiles = []
            for d in range(DT):
                r0 = d * 128
                home = nc.gpsimd if (d % 4 == 3) else nc.vector
                S = dq.tile([128, E], f16, tag="S", name="S")
                sx_t = dq.tile([128, E], f16, tag="sx", name="sx")
                Hh = dq.tile([128, E], f16, tag="H", name="H")
                nc.sync.dma_start(S[:, :], sg[r0:r0 + 128, :])
                nc.sync.dma_start(Hh[:, :], hg[r0:r0 + 128, :])
                nc.sync.dma_start(sx_t[:, :], sx[r0:r0 + 128, :])

                t2 = dq.tile([128, E], f16, tag="t2", name="t2")
                t4 = dq.tile([128, E], f16, tag="t4", name="t4")
                nc.scalar.activation(t2[:, :], S[:, :], AF.Square)
                nc.scalar.activation(t4[:, :], t2[:, :], AF.Square)
                An = dq.tile([128, E], f16, tag="An", name="An")
                Ad = dq.tile([128, E], f16, tag="Ad", name="Ad")
                nc.scalar.activation(An[:, :], S[:, :], AF.Copy,
                                     bias=float(CN[0]), scale=float(CN[1]))
                nc.scalar.activation(Ad[:, :], S[:, :], AF.Copy,
                                     bias=float(CD[0]), scale=float(CD[1]))
                prs = {}
                for nm, (c0, c1) in (("Bn", (CN[2], CN[3])), ("Bd", (CD[2], CD[3])),
                                     ("Cn", (CN[4], CN[5])), ("Cd", (CD[4], CD[5]))):
                    p = dq.tile([128, E], f16, tag=nm, name=nm)
                    nc.scalar.activation(p[:, :], S[:, :], AF.Copy,
                                         bias=float(c0), scale=float(c1))
                    prs[nm] = p

                def estrin(A, B, C, sufx):
                    u1 = dq.tile([128, E], f16, tag="u1" + sufx, name="u1" + sufx)
                    home.tensor_tensor(u1[:, :], t2[:, :], B[:, :], alu.mult)
                    u2 = dq.tile([128, E], f16, tag="u2" + sufx, name="u2" + sufx)
                    home.tensor_tensor(u2[:, :], t4[:, :], C[:, :], alu.mult)
                    v = dq.tile([128, E], f16, tag="v" + sufx, name="v" + sufx)
                    home.tensor_tensor(v[:, :], A[:, :], u1[:, :], alu.add)
                    P = dq.tile([128, E], f16, tag="P" + sufx, name="P" + sufx)
                    home.tensor_tensor(P[:, :], v[:, :], u2[:, :], alu.add)
                    return P

                Pn = estrin(An, prs["Bn"], prs["Cn"], "n")
                Pd = estrin(Ad, prs["Bd"], prs["Cd"], "d")

                uu = dq.tile([128, E], f16, tag="uu", name="uu")
                home.tensor_tensor(uu[:, :], Pd[:, :], Hh[:, :], alu.mult)
                vv = dq.tile([128, E], f16, tag="vv", name="vv")
                home.tensor_tensor(vv[:, :], Pn[:, :], uu[:, :], alu.add)

                w_t = wres.tile([128, E], f16, tag=f"w{d}", name=f"w{d}")
                home.tensor_tensor(w_t[:, :], vv[:, :], sx_t[:, :], alu.mult)
                wtiles.append(w_t)

            # ---- matmul: chunks of 8 m-tiles, K accumulation over 32 d-tiles
            for ch in range(MT // CHUNK):
                m0 = ch * XTM
                ps = [pp.tile([128, E], f32, tag="ps", name=f"ps{ch}_{i}")
                      for i in range(CHUNK)]
                for d in range(DT):
                    xt_t = xp.tile([128, XTM], f16, tag="xt", name="xt")
                    nc.sync.dma_start(xt_t[:, :], xt[d*128:(d+1)*128, m0:m0 + XTM])
                    for mt in range(CHUNK):
                        nc.tensor.matmul(
                            ps[mt][:, :],
                            xt_t[:, mt*128:(mt+1)*128],
                            wtiles[d][:, :],
                            start=(d == 0), stop=False)
                for mt in range(CHUNK):
                    nc.tensor.matmul(ps[mt][:, :], ones_t[:, :], bias_t[:, :],
                                     start=False, stop=True)
                    ot = op.tile([128, E], f32, tag="ot", name="ot")
                    nc.scalar.copy(ot[:, :], ps[mt][:, :])
                    r = m0 + mt * 128
                    nc.sync.dma_start(out[r:r + 128, :], ot[:, :])

    nc.compile()
    return nc


def _get_nc():
    if "nc" not in _NC_CACHE:
        _NC_CACHE["nc"] = _build_nc()
    return _NC_CACHE["nc"]


def _prep_inputs(x, kernel_quantized, kernel_scales, bias):
    X = np.asarray(x, dtype=np.float32).reshape(M, D_IN)
    xt_full = np.ascontiguousarray(X.astype(np.float16).T)  # [D_IN, M]
    kq = np.asarray(kernel_quantized).astype(np.uint8).reshape(D_IN, D_OUT // 2)
    codes = np.empty((D_IN, D_OUT), np.uint8)
    codes[:, 0::2] = kq >> 4
    codes[:, 1::2] = kq & 15
    sg_full = (((codes & 7).astype(np.float16) - np.float16(3.5)) * np.float16(0.25))
    hg_full = (codes >> 3).astype(np.float16)
    sc = np.asarray(kernel_scales, dtype=np.float32).reshape(D_IN, D_OUT // 64)
    sx_full = np.repeat(sc.astype(np.float16), 64, axis=1)          # [D_IN, D_OUT]
    b_full = np.asarray(bias, dtype=np.float32)

    in_maps = []
    for c in range(N_CORES):
        sl = slice(c * E, (c + 1) * E)
        in_maps.append({
            "xt": xt_full,
            "sg": np.ascontiguousarray(sg_full[:, sl]),
            "hg": np.ascontiguousarray(hg_full[:, sl]),
            "sx": np.ascontiguousarray(sx_full[:, sl]),
            "biasr": b_full[sl].astype(np.float16).reshape(1, E),
        })
    return in_maps


def kernel(x, kernel_quantized, kernel_scales, bias, _trace=False, _tmpdir=None):
    in_maps = _prep_inputs(x, kernel_quantized, kernel_scales, bias)
    nc = _get_nc()
    kwargs = {}
    if _trace:
        kwargs = {"trace": True, "tmpdir": _tmpdir}
    res = bass_utils.run_bass_kernel_spmd(
        nc, in_maps, core_ids=list(range(N_CORES)), **kwargs)
    out = np.concatenate([res.results[c]["out"] for c in range(N_CORES)], axis=1)
    out = np.ascontiguousarray(out).reshape(4, 2048, D_OUT).astype(np.float32)
    if _trace:
        return out, res
    return out
